# revision 13
# baseline (speedup 1.0000x reference)
"""Cross-attention kernel for Trainium2, sharded over 8 NeuronCores.

Problem (hardcoded): B=2, N=M=2048, query/context dim 1024, 8 heads x 64.
Sharding: core c -> (batch b=c//4, head-pair hp=c%4). Each core projects
q/k/v for its 2 heads (column-parallel), runs attention for those heads,
and computes a partial output projection (row-parallel over Wo). The host
sums the 4 partials per batch and adds the bias.

Device-side layout is fully transposed (feature dim on SBUF partitions):
  - qT/kT: [128 (2 heads x 64 dims), tokens]
  - sim computed transposed [keys, queries]; the softmax denominator
    comes from a ones-column in v via the attn@v accumulation.
  - exp mostly on ScalarE (table exp, scale fused); 4 of 16 key tiles
    per window use a Schraudolph-style bf16 fast exp on DVE/Pool -- one
    tensor_scalar affine op emitting int16 bits that reinterpret as
    bf16 ~= exp(x) -- to keep the Scalar engine off the critical path.
  - attn@v packs both heads in one PE pass via column tiling.

Schedule: kv/q projections for later windows are emitted in 4-matmul
slices between attention tiles so the PE never bursts long enough to
starve the exp stream. Inputs stream over the sync+gpsimd queues with
window 0 split across sync+scalar (scalar is idle pre-attention);
outputs write back in bf16 over rotating sync/gpsimd queues.
"""

import numpy as np
import ml_dtypes

B = 2
N = 2048  # query tokens per batch
M = 2048  # context tokens per batch
D = 1024  # query/context feature dim
HEADS = 8
DH = 64
INNER = 512
SCALE = DH**-0.5
P = 128
TW = 512  # token window
NKC = D // P  # contraction chunks for projections (8)
NJT = M // P  # key tiles (16)
NIW = N // TW  # query windows (4)
NW = M // TW  # key windows (4)

# Schraudolph bf16 exp: bits = round(x * SCALE * 2^7 * log2(e) + (127*2^7 - C))
SCH_A = SCALE * (2**7) * 1.4426950408889634
SCH_B = 127.0 * 2**7 - 8.0

_STATE = {}


def _build_nc():
    import concourse.bacc as bacc
    import concourse.tile as tile
    import concourse.mybir as mybir
    from concourse.masks import make_identity

    dt = mybir.dt
    bf16 = dt.bfloat16
    f32 = dt.float32
    i16 = dt.int16

    nc = bacc.Bacc("TRN2", target_bir_lowering=False, debug=False)

    xT = nc.dram_tensor("xT", [D, N], bf16, kind="ExternalInput").ap()
    ctxT = nc.dram_tensor("ctxT", [D, M], bf16, kind="ExternalInput").ap()
    wq = nc.dram_tensor("wq", [P, NKC, P], bf16, kind="ExternalInput").ap()
    wk = nc.dram_tensor("wk", [P, NKC, P], bf16, kind="ExternalInput").ap()
    wv = nc.dram_tensor("wv", [P, NKC, P], bf16, kind="ExternalInput").ap()
    wo = nc.dram_tensor("wo", [P, 2, 512], bf16, kind="ExternalInput").ap()
    outp = nc.dram_tensor("outp", [N, D], bf16, kind="ExternalOutput").ap()

    with tile.TileContext(nc) as tc:
        with (
            tc.tile_pool(name="const", bufs=1) as constp,
            tc.tile_pool(name="weights", bufs=1) as wpool,
            tc.tile_pool(name="persist", bufs=1) as persist,
            tc.tile_pool(name="qwin", bufs=4) as qpool,
            tc.tile_pool(name="attn", bufs=8) as apool,
            tc.tile_pool(name="vt", bufs=2) as vtpool,
            tc.tile_pool(name="norm", bufs=8) as npool,
            tc.tile_pool(name="evict", bufs=6) as epool,
            tc.tile_pool(name="psum_mm", bufs=2, space="PSUM") as psum_mm,
            tc.tile_pool(name="psum_sim", bufs=2, space="PSUM") as psum_sim,
            tc.tile_pool(name="psum_o", bufs=2, space="PSUM") as psum_o,
        ):
            identity = constp.tile([P, P], bf16)
            make_identity(nc, identity)
            ones = constp.tile([P, 64], bf16)
            nc.vector.memset(ones[:], 1.0)

            ctx_sb = persist.tile([P, NKC, M], bf16)
            x_sb = persist.tile([P, NKC, N], bf16)

            def load_half(q, dst, src, w, half):
                wsl = slice(w * TW, (w + 1) * TW)
                for kc in range(4 * half, 4 * half + 4):
                    q.dma_start(dst[:, kc, wsl], src[kc * P : (kc + 1) * P, wsl])

            # latency-ordered input streaming; scalar queue helps only with
            # window 0 (it must be free again by the time exp starts)
            wq_sb = wpool.tile([P, NKC, P], bf16)
            nc.sync.dma_start(wq_sb[:], wq[:])
            wk_sb = wpool.tile([P, NKC, P], bf16)
            nc.gpsimd.dma_start(wk_sb[:], wk[:])
            load_half(nc.scalar, ctx_sb, ctxT, 0, 0)
            load_half(nc.gpsimd, ctx_sb, ctxT, 0, 1)
            load_half(nc.sync, x_sb, xT, 0, 0)
            load_half(nc.scalar, x_sb, xT, 0, 1)
            wv_sb = wpool.tile([P, NKC, P], bf16)
            nc.gpsimd.dma_start(wv_sb[:], wv[:])
            load_half(nc.sync, ctx_sb, ctxT, 1, 0)
            load_half(nc.gpsimd, ctx_sb, ctxT, 1, 1)
            wo_sb = wpool.tile([P, 2, 512], bf16)
            nc.sync.dma_start(wo_sb[:], wo[:])
            load_half(nc.gpsimd, ctx_sb, ctxT, 2, 0)
            load_half(nc.sync, ctx_sb, ctxT, 2, 1)
            load_half(nc.gpsimd, ctx_sb, ctxT, 3, 0)
            load_half(nc.sync, ctx_sb, ctxT, 3, 1)
            load_half(nc.gpsimd, x_sb, xT, 1, 0)
            load_half(nc.sync, x_sb, xT, 1, 1)
            load_half(nc.gpsimd, x_sb, xT, 2, 0)
            load_half(nc.sync, x_sb, xT, 2, 1)
            load_half(nc.gpsimd, x_sb, xT, 3, 0)
            load_half(nc.sync, x_sb, xT, 3, 1)

            kTw = [
                persist.tile([P, TW], bf16, name=f"kTw{w}", tag=f"kTw{w}")
                for w in range(NW)
            ]
            v3w = [
                persist.tile([P, TW // P, 130], bf16, name=f"v3w{w}", tag=f"v3w{w}")
                for w in range(NW)
            ]
            for w in range(NW):
                nc.vector.memset(v3w[w][:, :, 0:1], 1.0)
                nc.vector.memset(v3w[w][:, :, 65:66], 1.0)

            def qproj(iw):
                iwsl = slice(iw * TW, (iw + 1) * TW)
                psq = psum_mm.tile([P, TW], f32, tag="mm")
                for kc in range(NKC):
                    nc.tensor.matmul(
                        psq[:], wq_sb[:, kc, :], x_sb[:, kc, iwsl],
                        start=(kc == 0), stop=(kc == NKC - 1),
                    )
                qw = qpool.tile([P, TW], bf16, tag="qw")
                nc.vector.tensor_copy(qw[:], psq[:])
                return qw

            kvstate = {}

            def kvproj_part(jw, part):
                # part 0..3: two contraction chunks of k and v each; the
                # last part evicts kT and transposes v into v3w.
                jwsl = slice(jw * TW, (jw + 1) * TW)
                if part == 0:
                    kvstate[jw] = (
                        psum_mm.tile([P, TW], f32, name=f"psk{jw}", tag="mm"),
                        psum_mm.tile([P, TW], f32, name=f"psv{jw}", tag="mm"),
                    )
                psk, psv = kvstate[jw]
                for kc in range(2 * part, 2 * part + 2):
                    nc.tensor.matmul(
                        psk[:], wk_sb[:, kc, :], ctx_sb[:, kc, jwsl],
                        start=(kc == 0), stop=(kc == NKC - 1),
                    )
                    nc.tensor.matmul(
                        psv[:], wv_sb[:, kc, :], ctx_sb[:, kc, jwsl],
                        start=(kc == 0), stop=(kc == NKC - 1),
                    )
                if part == 3:
                    nc.vector.tensor_copy(kTw[jw][:], psk[:])
                    vt = vtpool.tile([P, TW], bf16, tag="vt")
                    nc.vector.tensor_copy(vt[:], psv[:])
                    for t in range(TW // P):
                        pst = psum_mm.tile([P, P], bf16, tag="mm")
                        nc.tensor.transpose(
                            pst[:], vt[:, t * P : (t + 1) * P], identity[:]
                        )
                        nc.vector.tensor_copy(v3w[jw][:, t, 1:65], pst[:, 0:64])
                        nc.vector.tensor_copy(v3w[jw][:, t, 66:130], pst[:, 64:128])

            # warm the PE p-state during the input-DMA prologue so the
            # first projections run at full clock (ramp needs ~3us busy)
            for wi in range(24):
                wt = psum_sim.tile([P, P], bf16, name=f"warm{wi}", tag="sim")
                nc.tensor.transpose(wt[:], identity[:], identity[:])

            # prologue: q window 0, k/v window 0
            qws = [None] * NIW
            qws[0] = qproj(0)
            for part in range(4):
                kvproj_part(0, part)

            outq = [nc.sync, nc.gpsimd]

            # attention + pipelined projections
            for iw in range(NIW):
                qw = qws[iw]
                o_psA = psum_o.tile([65, TW], f32, tag="o")
                o_psB = psum_o.tile([65, TW], f32, tag="o")
                for jt in range(NJT):
                    jw, t = jt // (TW // P), jt % (TW // P)
                    jsl = slice(t * P, (t + 1) * P)
                    first, last = jt == 0, jt == NJT - 1
                    s2 = psum_sim.tile([P, 2 * TW], f32, tag="sim")
                    nc.tensor.matmul(
                        s2[:, 0:TW], kTw[jw][0:64, jsl], qw[0:64, :],
                        skip_group_check=True,
                    )
                    nc.tensor.matmul(
                        s2[:, TW : 2 * TW], kTw[jw][64:128, jsl], qw[64:128, :],
                        skip_group_check=True,
                    )
                    # gpsimd cannot read PSUM, so fast-exp tiles go to DVE
                    eng = nc.vector if jt in (2, 6, 10, 14) else None
                    if eng is None:
                        a2 = apool.tile([P, 2 * TW], bf16, tag="a")
                        nc.scalar.activation(
                            a2[:], s2[:], mybir.ActivationFunctionType.Exp,
                            scale=SCALE,
                        )
                        aA, aB = a2[:, 0:TW], a2[:, TW : 2 * TW]
                    else:
                        ai = apool.tile([P, 2 * TW], i16, tag="a")
                        eng.tensor_scalar(
                            ai[:], s2[:], SCH_A, SCH_B,
                            mybir.AluOpType.mult, mybir.AluOpType.add,
                        )
                        aA = ai[:, 0:TW].bitcast(bf16)
                        aB = ai[:, TW : 2 * TW].bitcast(bf16)
                    nc.tensor.matmul(
                        o_psA[:], v3w[jw][:, t, 0:65], aA,
                        start=first, stop=last, skip_group_check=True,
                    )
                    nc.tensor.matmul(
                        o_psB[:], v3w[jw][:, t, 65:130], aB,
                        start=first, stop=last, skip_group_check=True,
                    )
                    # pipeline later projection windows into the stream
                    if iw == 0 and jt < 12:
                        kvproj_part(jt // 4 + 1, jt % 4)
                    if jt == 13 and iw + 1 < NIW:
                        qws[iw + 1] = qproj(iw + 1)

                # normalize. S_h sits in row 0 of each accumulator.
                evA = npool.tile([65, TW], bf16, tag="evA")
                nc.vector.tensor_copy(evA[:], o_psA[:])
                evB = npool.tile([65, TW], bf16, tag="evB")
                nc.vector.tensor_copy(evB[:], o_psB[:])
                # broadcast S across partitions (rows 0-63 = S_A, 64-127 = S_B)
                bc_ps = psum_mm.tile([P, TW], f32, tag="mm")
                nc.tensor.matmul(bc_ps[0:64, :], ones[0:1, 0:64], evA[0:1, :])
                nc.tensor.matmul(bc_ps[64:128, :], ones[0:1, 0:64], evB[0:1, :])
                bc_sb = npool.tile([P, TW], f32, tag="bc")
                nc.vector.reciprocal_approx_fast(bc_sb[:], bc_ps[:])
                # lane-shift unnormalized o into a single [128, TW] tile
                ao_u = npool.tile([P, TW], bf16, tag="aou")
                nc.sync.dma_start(ao_u[0:64, :], evA[1:65, :])
                # keep gpsimd's last DMA early; scalar is done with exps here
                (nc.scalar if iw == NIW - 1 else nc.gpsimd).dma_start(
                    ao_u[64:128, :], evB[1:65, :]
                )
                ao = npool.tile([P, TW], bf16, tag="ao")
                nc.vector.tensor_mul(ao[:], ao_u[:], bc_sb[:])
                # partial output projection: [tokens, out_feat]
                for it in range(TW // P):
                    r0 = iw * TW + it * P
                    for fc in range(2):
                        op_ps = psum_mm.tile([P, 512], f32, tag="mm")
                        nc.tensor.matmul(
                            op_ps[:], ao[:, it * P : (it + 1) * P], wo_sb[:, fc, :]
                        )
                        ev = epool.tile([P, 512], bf16, tag="ev")
                        nc.vector.tensor_copy(ev[:], op_ps[:])
                        # last window drains on sync only: the gpsimd swdge
                        # queue is slow to drain and would pad the epilogue
                        q = nc.sync if iw == NIW - 1 else outq[(it * 2 + fc) % 2]
                        q.dma_start(
                            outp[r0 : r0 + P, fc * 512 : (fc + 1) * 512], ev[:]
                        )

    nc.compile()
    return nc


def _get_nc():
    if "nc" not in _STATE:
        _STATE["nc"] = _build_nc()
    return _STATE["nc"]


def _make_in_maps(x, context, Wq, Wk, Wv, Wo):
    bf = ml_dtypes.bfloat16

    def wslice(W, hp):
        # [1024, 128] -> [p, kc, m] with k = kc*128 + p
        s = W[:, hp * P : (hp + 1) * P]
        return np.ascontiguousarray(
            s.reshape(NKC, P, P).transpose(1, 0, 2)
        ).astype(bf)

    xTs = [np.ascontiguousarray(x[b].T).astype(bf) for b in range(B)]
    cTs = [np.ascontiguousarray(context[b].T).astype(bf) for b in range(B)]
    in_maps = []
    for c in range(8):
        b, hp = c // 4, c % 4
        in_maps.append(
            {
                "xT": xTs[b],
                "ctxT": cTs[b],
                "wq": wslice(Wq, hp),
                "wk": wslice(Wk, hp),
                "wv": wslice(Wv, hp),
                "wo": np.ascontiguousarray(
                    Wo[hp * P : (hp + 1) * P, :].reshape(P, 2, 512)
                ).astype(bf),
            }
        )
    return in_maps


def kernel(x, context, Wq, Wk, Wv, Wo, bo, _spmd_kwargs=None):
    from concourse.bass_utils import run_bass_kernel_spmd

    nc = _get_nc()
    in_maps = _make_in_maps(x, context, Wq, Wk, Wv, Wo)
    res = run_bass_kernel_spmd(
        nc, in_maps, core_ids=list(range(8)), **(_spmd_kwargs or {})
    )
    _STATE["last_result"] = res
    outs = [np.asarray(r["outp"], dtype=np.float32) for r in res.results]
    out = np.empty((B, N, D), np.float32)
    for b in range(B):
        out[b] = outs[4 * b] + outs[4 * b + 1] + outs[4 * b + 2] + outs[4 * b + 3]
        out[b] += bo.astype(np.float32)
    return out


# revision 25
# speedup vs baseline: 1.2666x; 1.2666x over previous
"""Cross-attention kernel for Trainium2, sharded over 8 NeuronCores.

Problem (hardcoded): B=2, N=M=2048, query/context dim 1024, 8 heads x 64.
Sharding: core c -> (batch b=c//4, head-pair hp=c%4). Each core projects
q/k/v for its 2 heads (column-parallel), runs attention for those heads,
and computes a partial output projection (row-parallel over Wo). The host
sums the 4 partials per batch and adds the bias.

Device-side layout is fully transposed (feature dim on SBUF partitions):
  - qT/kT: [128 (2 heads x 64 dims), tokens]
  - sim computed transposed [keys, queries]; the softmax denominator
    comes from a ones-column in v via the attn@v accumulation.
  - exp mostly on ScalarE (table exp, scale fused); 4 of 16 key tiles
    per window use a Schraudolph-style bf16 fast exp on DVE/Pool -- one
    tensor_scalar affine op emitting int16 bits that reinterpret as
    bf16 ~= exp(x) -- to keep the Scalar engine off the critical path.
  - attn@v packs both heads in one PE pass via column tiling.

Schedule: kv/q projections for later windows are emitted in 4-matmul
slices between attention tiles so the PE never bursts long enough to
starve the exp stream. Inputs stream over the sync+gpsimd queues with
window 0 split across sync+scalar (scalar is idle pre-attention);
outputs write back in bf16 over rotating sync/gpsimd queues.
"""

import numpy as np
import ml_dtypes

B = 2
N = 2048  # query tokens per batch
M = 2048  # context tokens per batch
D = 1024  # query/context feature dim
HEADS = 8
DH = 64
INNER = 512
SCALE = DH**-0.5
P = 128
TW = 512  # token window
NKC = D // P  # contraction chunks for projections (8)
NJT = M // P  # key tiles (16)
NIW = N // TW  # query windows (4)
NW = M // TW  # key windows (4)

# Schraudolph bf16 exp: bits = round(x * SCALE * 2^7 * log2(e) + (127*2^7 - C))
SCH_A = SCALE * (2**7) * 1.4426950408889634
SCH_B = 127.0 * 2**7 - 8.0

_STATE = {}


def _build_nc():
    import concourse.bacc as bacc
    import concourse.tile as tile
    import concourse.mybir as mybir
    from concourse.masks import make_identity

    dt = mybir.dt
    bf16 = dt.bfloat16
    f32 = dt.float32
    i16 = dt.int16

    nc = bacc.Bacc("TRN2", target_bir_lowering=False, debug=False)

    xT = nc.dram_tensor("xT", [D, N], bf16, kind="ExternalInput").ap()
    ctxT = nc.dram_tensor("ctxT", [D, M], bf16, kind="ExternalInput").ap()
    wq = nc.dram_tensor("wq", [P, NKC, P], bf16, kind="ExternalInput").ap()
    wk = nc.dram_tensor("wk", [P, NKC, P], bf16, kind="ExternalInput").ap()
    wv = nc.dram_tensor("wv", [P, NKC, P], bf16, kind="ExternalInput").ap()
    wo = nc.dram_tensor("wo", [P, 2, 512], bf16, kind="ExternalInput").ap()
    outp = nc.dram_tensor("outp", [N, D], bf16, kind="ExternalOutput").ap()

    with tile.TileContext(nc) as tc:
        with (
            tc.tile_pool(name="const", bufs=1) as constp,
            tc.tile_pool(name="weights", bufs=1) as wpool,
            tc.tile_pool(name="persist", bufs=1) as persist,
            tc.tile_pool(name="qwin", bufs=4) as qpool,
            tc.tile_pool(name="attn", bufs=8) as apool,
            tc.tile_pool(name="vt", bufs=2) as vtpool,
            tc.tile_pool(name="norm", bufs=8) as npool,
            tc.tile_pool(name="psum_mm", bufs=2, space="PSUM") as psum_mm,
            tc.tile_pool(name="psum_sim", bufs=2, space="PSUM") as psum_sim,
            tc.tile_pool(name="psum_o", bufs=2, space="PSUM") as psum_o,
        ):
            identity = constp.tile([P, P], bf16)
            make_identity(nc, identity)
            ones = constp.tile([P, 64], bf16)
            nc.vector.memset(ones[:], 1.0)

            ctx_sb = persist.tile([P, NKC, M], bf16)
            x_sb = persist.tile([P, NKC, N], bf16)

            def load_half(q, dst, src, w, half):
                wsl = slice(w * TW, (w + 1) * TW)
                for kc in range(4 * half, 4 * half + 4):
                    q.dma_start(dst[:, kc, wsl], src[kc * P : (kc + 1) * P, wsl])

            # latency-ordered input streaming; scalar queue helps only with
            # window 0 (it must be free again by the time exp starts)
            wq_sb = wpool.tile([P, NKC, P], bf16)
            nc.sync.dma_start(wq_sb[:], wq[:])
            wk_sb = wpool.tile([P, NKC, P], bf16)
            nc.gpsimd.dma_start(wk_sb[:], wk[:])
            load_half(nc.scalar, ctx_sb, ctxT, 0, 0)
            load_half(nc.gpsimd, ctx_sb, ctxT, 0, 1)
            load_half(nc.sync, x_sb, xT, 0, 0)
            load_half(nc.scalar, x_sb, xT, 0, 1)
            wv_sb = wpool.tile([P, NKC, P], bf16)
            nc.gpsimd.dma_start(wv_sb[:], wv[:])
            load_half(nc.sync, ctx_sb, ctxT, 1, 0)
            load_half(nc.gpsimd, ctx_sb, ctxT, 1, 1)
            wo_sb = wpool.tile([P, 2, 512], bf16)
            nc.sync.dma_start(wo_sb[:], wo[:])
            load_half(nc.gpsimd, ctx_sb, ctxT, 2, 0)
            load_half(nc.sync, ctx_sb, ctxT, 2, 1)
            load_half(nc.gpsimd, ctx_sb, ctxT, 3, 0)
            load_half(nc.sync, ctx_sb, ctxT, 3, 1)
            load_half(nc.gpsimd, x_sb, xT, 1, 0)
            load_half(nc.sync, x_sb, xT, 1, 1)
            load_half(nc.gpsimd, x_sb, xT, 2, 0)
            load_half(nc.sync, x_sb, xT, 2, 1)
            load_half(nc.gpsimd, x_sb, xT, 3, 0)
            load_half(nc.sync, x_sb, xT, 3, 1)

            kTw = [
                persist.tile([P, TW], bf16, name=f"kTw{w}", tag=f"kTw{w}")
                for w in range(NW)
            ]
            v3w = [
                persist.tile([P, TW // P, 130], bf16, name=f"v3w{w}", tag=f"v3w{w}")
                for w in range(NW)
            ]
            for w in range(NW):
                nc.vector.memset(v3w[w][:, :, 0:1], 1.0)
                nc.vector.memset(v3w[w][:, :, 65:66], 1.0)

            def qproj(iw):
                iwsl = slice(iw * TW, (iw + 1) * TW)
                psq = psum_mm.tile([P, TW], f32, tag="mm")
                for kc in range(NKC):
                    nc.tensor.matmul(
                        psq[:], wq_sb[:, kc, :], x_sb[:, kc, iwsl],
                        start=(kc == 0), stop=(kc == NKC - 1),
                    )
                qw = qpool.tile([P, TW], bf16, tag="qw")
                nc.vector.tensor_copy(qw[:], psq[:])
                return qw

            kvstate = {}

            def kvproj_part(jw, part):
                # part 0..3: two contraction chunks of k and v each; the
                # last part evicts kT and transposes v into v3w.
                jwsl = slice(jw * TW, (jw + 1) * TW)
                if part == 0:
                    kvstate[jw] = (
                        psum_mm.tile([P, TW], f32, name=f"psk{jw}", tag="mm"),
                        psum_mm.tile([P, TW], f32, name=f"psv{jw}", tag="mm"),
                    )
                psk, psv = kvstate[jw]
                for kc in range(2 * part, 2 * part + 2):
                    nc.tensor.matmul(
                        psk[:], wk_sb[:, kc, :], ctx_sb[:, kc, jwsl],
                        start=(kc == 0), stop=(kc == NKC - 1),
                    )
                    nc.tensor.matmul(
                        psv[:], wv_sb[:, kc, :], ctx_sb[:, kc, jwsl],
                        start=(kc == 0), stop=(kc == NKC - 1),
                    )
                if part == 3:
                    # split psum evictions across scalar+vector queues
                    nc.scalar.copy(kTw[jw][:], psk[:])
                    vt = vtpool.tile([P, TW], bf16, tag="vt")
                    nc.vector.tensor_copy(vt[:], psv[:])
                    for t in range(TW // P):
                        pst = psum_mm.tile([P, P], bf16, tag="mm")
                        nc.tensor.transpose(
                            pst[:], vt[:, t * P : (t + 1) * P], identity[:]
                        )
                        if t % 2 == 0:
                            nc.vector.tensor_copy(v3w[jw][:, t, 1:65], pst[:, 0:64])
                            nc.vector.tensor_copy(
                                v3w[jw][:, t, 66:130], pst[:, 64:128]
                            )
                        else:
                            nc.scalar.copy(v3w[jw][:, t, 1:65], pst[:, 0:64])
                            nc.scalar.copy(v3w[jw][:, t, 66:130], pst[:, 64:128])

            # prologue: q window 0, k/v window 0
            qws = [None] * NIW
            qws[0] = qproj(0)
            for part in range(4):
                kvproj_part(0, part)

            outq = [nc.sync, nc.gpsimd]

            # attention + pipelined projections
            for iw in range(NIW):
                qw = qws[iw]
                o_psA = psum_o.tile([65, TW], f32, tag="o")
                o_psB = psum_o.tile([65, TW], f32, tag="o")
                for jt in range(NJT):
                    jw, t = jt // (TW // P), jt % (TW // P)
                    jsl = slice(t * P, (t + 1) * P)
                    first, last = jt == 0, jt == NJT - 1
                    s2 = psum_sim.tile([P, 2 * TW], f32, tag="sim")
                    nc.tensor.matmul(
                        s2[:, 0:TW], kTw[jw][0:64, jsl], qw[0:64, :],
                        skip_group_check=True,
                    )
                    nc.tensor.matmul(
                        s2[:, TW : 2 * TW], kTw[jw][64:128, jsl], qw[64:128, :],
                        skip_group_check=True,
                    )
                    # gpsimd cannot read PSUM, so fast-exp tiles go to DVE;
                    # window 0 keeps DVE free for the k/v eviction copies
                    eng = nc.vector if iw > 0 and jt in (2, 5, 8, 11, 14) else None
                    if eng is None:
                        a2 = apool.tile([P, 2 * TW], bf16, tag="a")
                        nc.scalar.activation(
                            a2[:], s2[:], mybir.ActivationFunctionType.Exp,
                            scale=SCALE,
                        )
                        aA, aB = a2[:, 0:TW], a2[:, TW : 2 * TW]
                    else:
                        ai = apool.tile([P, 2 * TW], i16, tag="a")
                        eng.tensor_scalar(
                            ai[:], s2[:], SCH_A, SCH_B,
                            mybir.AluOpType.mult, mybir.AluOpType.add,
                        )
                        aA = ai[:, 0:TW].bitcast(bf16)
                        aB = ai[:, TW : 2 * TW].bitcast(bf16)
                    nc.tensor.matmul(
                        o_psA[:], v3w[jw][:, t, 0:65], aA,
                        start=first, stop=last, skip_group_check=True,
                    )
                    nc.tensor.matmul(
                        o_psB[:], v3w[jw][:, t, 65:130], aB,
                        start=first, stop=last, skip_group_check=True,
                    )
                    # pipeline later projection windows into the stream
                    # (two 8-matmul slices per kv window, early enough that
                    # kv window j is ready well before its first sim)
                    if iw == 0 and jt in (0, 1, 4, 5, 8, 9):
                        jwn = jt // 4 + 1
                        kvproj_part(jwn, 2 * (jt % 4))
                        kvproj_part(jwn, 2 * (jt % 4) + 1)
                    if jt == 13 and iw + 1 < NIW:
                        qws[iw + 1] = qproj(iw + 1)

                # normalize. S_h sits in row 0 of each accumulator.
                evA = npool.tile([65, TW], bf16, tag="evA")
                nc.vector.tensor_copy(evA[:], o_psA[:])
                evB = npool.tile([65, TW], bf16, tag="evB")
                nc.scalar.copy(evB[:], o_psB[:])
                # broadcast S across partitions (rows 0-63 = S_A, 64-127 = S_B)
                bc_ps = psum_mm.tile([P, TW], f32, tag="mm")
                nc.tensor.matmul(bc_ps[0:64, :], ones[0:1, 0:64], evA[0:1, :])
                nc.tensor.matmul(bc_ps[64:128, :], ones[0:1, 0:64], evB[0:1, :])
                bc_sb = npool.tile([P, TW], f32, tag="bc")
                nc.vector.reciprocal_approx_fast(bc_sb[:], bc_ps[:])
                # lane-shift unnormalized o into a single [128, TW] tile
                ao_u = npool.tile([P, TW], bf16, tag="aou")
                nc.sync.dma_start(ao_u[0:64, :], evA[1:65, :])
                # keep gpsimd's last DMA early; scalar is done with exps here
                (nc.scalar if iw == NIW - 1 else nc.gpsimd).dma_start(
                    ao_u[64:128, :], evB[1:65, :]
                )
                ao = npool.tile([P, TW], bf16, tag="ao")
                nc.vector.tensor_mul(ao[:], ao_u[:], bc_sb[:])
                # partial output projection: [tokens, out_feat]
                for it in range(TW // P):
                    r0 = iw * TW + it * P
                    for fc in range(2):
                        op_ps = psum_mm.tile([P, 512], f32, tag="mm")
                        nc.tensor.matmul(
                            op_ps[:], ao[:, it * P : (it + 1) * P], wo_sb[:, fc, :]
                        )
                        ev = npool.tile([P, 512], bf16, tag="ev")
                        if fc == 0:
                            nc.vector.tensor_copy(ev[:], op_ps[:])
                        else:
                            nc.scalar.copy(ev[:], op_ps[:])
                        # last window drains on sync+scalar: the gpsimd swdge
                        # queue is slow to drain and would pad the epilogue
                        if iw == NIW - 1:
                            q = nc.sync if fc == 0 else nc.scalar
                        else:
                            q = outq[(it * 2 + fc) % 2]
                        q.dma_start(
                            outp[r0 : r0 + P, fc * 512 : (fc + 1) * 512], ev[:]
                        )

    nc.compile()
    return nc


def _get_nc():
    if "nc" not in _STATE:
        _STATE["nc"] = _build_nc()
    return _STATE["nc"]


def _make_in_maps(x, context, Wq, Wk, Wv, Wo):
    bf = ml_dtypes.bfloat16

    def wslice(W, hp):
        # [1024, 128] -> [p, kc, m] with k = kc*128 + p
        s = W[:, hp * P : (hp + 1) * P]
        return np.ascontiguousarray(
            s.reshape(NKC, P, P).transpose(1, 0, 2)
        ).astype(bf)

    xTs = [np.ascontiguousarray(x[b].T).astype(bf) for b in range(B)]
    cTs = [np.ascontiguousarray(context[b].T).astype(bf) for b in range(B)]
    in_maps = []
    for c in range(8):
        b, hp = c // 4, c % 4
        in_maps.append(
            {
                "xT": xTs[b],
                "ctxT": cTs[b],
                "wq": wslice(Wq, hp),
                "wk": wslice(Wk, hp),
                "wv": wslice(Wv, hp),
                "wo": np.ascontiguousarray(
                    Wo[hp * P : (hp + 1) * P, :].reshape(P, 2, 512)
                ).astype(bf),
            }
        )
    return in_maps


def kernel(x, context, Wq, Wk, Wv, Wo, bo, _spmd_kwargs=None):
    from concourse.bass_utils import run_bass_kernel_spmd

    nc = _get_nc()
    in_maps = _make_in_maps(x, context, Wq, Wk, Wv, Wo)
    res = run_bass_kernel_spmd(
        nc, in_maps, core_ids=list(range(8)), **(_spmd_kwargs or {})
    )
    _STATE["last_result"] = res
    outs = [np.asarray(r["outp"], dtype=np.float32) for r in res.results]
    out = np.empty((B, N, D), np.float32)
    for b in range(B):
        out[b] = outs[4 * b] + outs[4 * b + 1] + outs[4 * b + 2] + outs[4 * b + 3]
        out[b] += bo.astype(np.float32)
    return out
